# revision 31
# baseline (speedup 1.0000x reference)
"""Trainium2 Bass kernel: batched forward kinematics (nn_DiffKin), v2.

Computes, for each batch element b and frame n:
    W[b, n] = prod_{i<=n} ( O_i @ M_i(angle_i(b)) )        (4x4 transforms)

v2 architecture (vs v1 which did L-build on GpSimd and an fp32 chain on DVE):
  * fp16 end-to-end on device (validated on host: rel_l2 ~2.0e-3 vs the
    fp64 reference; harness gate is 2e-2). Host upconverts to fp32.
  * Coefficient planes built s-major: theta is transposed on the (otherwise
    idle) TensorEngine into a [125, B_core] coefficient tile whose rows are
    [u-rows(61 rev) | pri-x(2) | w-rows(61) | ones]; ScalarE then applies
    Sin / Abs+Sin with per-partition (scale, bias) in 3 big ops.
  * Per-frame local transforms L_n = A'_n + sin(x)B_n + cos(x)(-C_n) for ALL
    frames come from PE matmuls: lhsT = coef column [125, 128] (stationary,
    one load per batch column), rhs = host-built block-sparse table
    [125, 64*12] fp16, split into 4 frame-groups of 16 so the chain can
    start after group 0. PSUM results are evacuated to an SBUF L table
    [P, nf, 12, q] fp16 by ScalarE.
  * The sequential chain W_n = W_{n-1} @ L_n runs on DVE (+ optional GpSimd
    q-slice) in fp16: 5 tensor_tensor ops on the 3x3 rotation block (2x DVE
    mode) + 3 scalar_tensor_tensor ops for the (constant-per-frame) t-column.
  * Output staged fp16, DMA'd out as fp16 (halves DMA bytes); host astype.
"""

import os
import sys

import numpy as np

for _p in ("/opt/trn_rl_repo", "/root/.axon_site/_ro/trn_rl_repo"):
    if os.path.isdir(_p) and _p not in sys.path:
        sys.path.append(_p)

import concourse.bass as bass  # noqa: E402
import concourse.tile as tile  # noqa: E402
from concourse import bacc, masks, mybir  # noqa: E402
from concourse.bass_utils import run_bass_kernel_spmd  # noqa: E402

F32 = mybir.dt.float32
F16 = mybir.dt.float16
AF = mybir.ActivationFunctionType
OP = mybir.AluOpType

N_CORES = 8
P = 128          # SBUF partitions
CHUNK = 4        # frames per output staging chunk
FG = 16          # frames per matmul group
QPAIR = 2        # batch-columns per PSUM evac group
# chain q-split: DVE gets [0:QS], GpSimd [QS:64]
QS = int(os.environ.get("FK_QS", "52"))

last_results = None
last_in_maps = None
_program_cache = {}


def _skew(a):
    x, y, z = a
    return np.array([[0.0, -z, y], [z, 0.0, -x], [-y, x, 0.0]], dtype=np.float64)


# --------------------------------------------------------------------------
# Host-side specialization
# --------------------------------------------------------------------------

def _frame_specs(all_axes, all_origins, mimic_multipliers, mimic_offsets,
                 ctrlable_indices, mimic_dst_indices, mimic_src_indices,
                 joint_types):
    """Fold structural inputs into per-frame specs.

    Returns frames: list of dicts per frame:
        kind: 'rev' | 'pri' | 'const'
        src, mult, off  (var kinds; angle_n(b) = mult*theta[b,src] + off)
        A, B, C : constant 4x4 float64 blocks (B/C only for var kinds)
    """
    axes = np.asarray(all_axes, dtype=np.float64)
    origins = np.asarray(all_origins, dtype=np.float64)
    nf = origins.shape[0]
    types = np.asarray(joint_types).astype(np.int64)
    ctrl = np.asarray(ctrlable_indices).astype(np.int64)
    mdst = np.asarray(mimic_dst_indices).astype(np.int64)
    msrc = np.asarray(mimic_src_indices).astype(np.int64)
    mmul = np.asarray(mimic_multipliers, dtype=np.float64)
    moff = np.asarray(mimic_offsets, dtype=np.float64)

    bottom = origins[:, 3, :]
    affine = bool(np.all(np.abs(bottom - np.array([0.0, 0.0, 0.0, 1.0])) < 1e-6))
    assert affine, "v2 kernel requires affine origins"

    src = [None] * nf
    mult = [0.0] * nf
    off = [0.0] * nf
    for j, ci in enumerate(ctrl):
        src[int(ci)] = j
        mult[int(ci)] = 1.0
        off[int(ci)] = 0.0
    pre = (list(src), list(mult), list(off))
    for d, s, m, o in zip(mdst, msrc, mmul, moff):
        d, s = int(d), int(s)
        if pre[0][s] is not None:
            src[d] = pre[0][s]
            mult[d] = float(m) * pre[1][s]
            off[d] = float(m) * pre[2][s] + float(o)
        else:
            src[d] = None
            mult[d] = 0.0
            off[d] = float(o)

    frames = []
    for n in range(nf):
        O4 = origins[n]
        t = int(types[n])
        if t == 1:
            r = float(np.linalg.norm(axes[n]))
            if r < 1e-20 or src[n] is None:
                # degenerate or constant-angle revolute -> constant frame
                if src[n] is None and r >= 1e-20:
                    K4 = np.zeros((4, 4))
                    K4[:3, :3] = _skew(axes[n] / r)
                    a = r * off[n]
                    M = (O4 + np.sin(a) * (O4 @ K4)
                         + (1.0 - np.cos(a)) * (O4 @ K4 @ K4))
                else:
                    M = O4
                frames.append(dict(kind="const", A=M))
            else:
                K4 = np.zeros((4, 4))
                K4[:3, :3] = _skew(axes[n] / r)
                frames.append(dict(kind="rev", src=src[n],
                                   mult=r * mult[n], off=r * off[n],
                                   A=O4, B=O4 @ K4, C=O4 @ K4 @ K4))
        elif t == 2:
            T4 = np.zeros((4, 4))
            T4[:3, 3] = axes[n]
            B = O4 @ T4
            if src[n] is None:
                frames.append(dict(kind="const", A=O4 + off[n] * B))
            else:
                frames.append(dict(kind="pri", src=src[n],
                                   mult=mult[n], off=off[n], A=O4, B=B))
        else:
            frames.append(dict(kind="const", A=O4))
    return frames


def _host_spec(frames):
    """Row layout, table, per-row consts, chain immediates."""
    nf = len(frames)
    rev = [n for n, f in enumerate(frames) if f["kind"] == "rev"]
    pri = [n for n, f in enumerate(frames) if f["kind"] == "pri"]
    nu, npri = len(rev), len(pri)
    # rows: [0..nu) rev-u | [nu..nu+npri) pri-x | pad | [w0..w0+nu) rev-w
    #       | ones row.  w0 is 32-aligned: engine ops on the w rows must
    #       start at a partition base that is a multiple of 32.
    w0 = ((nu + npri + 31) // 32) * 32
    ones_row = w0 + nu
    K = ones_row + 1
    assert K <= P

    srccol = [None] * (ones_row)       # theta column feeding each x row
    mult_arr = np.ones(P, np.float64)
    off_arr = np.zeros(P, np.float64)
    u_row = {}
    w_row = {}
    for i, n in enumerate(rev):
        f = frames[n]
        u_row[n] = i
        w_row[n] = w0 + i
        srccol[i] = f["src"]
        srccol[w0 + i] = f["src"]
        mult_arr[i] = mult_arr[w0 + i] = f["mult"]
        off_arr[i] = off_arr[w0 + i] = f["off"]
    for i, n in enumerate(pri):
        u_row[n] = nu + i
        srccol[nu + i] = frames[n]["src"]
        # pri rows get (mult, off) applied batch-major before the transpose

    table = np.zeros((P, nf * 12), np.float64)
    tcols = [None] * nf               # chain t-col immediates (rev/const)
    for n, f in enumerate(frames):
        cols = slice(12 * n, 12 * n + 12)
        if f["kind"] == "rev":
            Ap = f["A"] + f["C"]
            table[ones_row, cols] = Ap[:3, :].reshape(-1)
            table[u_row[n], cols] = f["B"][:3, :].reshape(-1)
            table[w_row[n], cols] = (-f["C"])[:3, :].reshape(-1)
            tcols[n] = tuple(float(v) for v in Ap[:3, 3])
        elif f["kind"] == "pri":
            table[ones_row, cols] = f["A"][:3, :].reshape(-1)
            table[u_row[n], cols] = f["B"][:3, :].reshape(-1)
        else:
            table[ones_row, cols] = f["A"][:3, :].reshape(-1)
            tcols[n] = tuple(float(v) for v in f["A"][:3, 3])

    # contiguous copy runs for theta_dup build: (dst0, src0, len);
    # rows with srccol None (pri rows, pad rows) are handled separately.
    runs = []
    zero_rows = [r for r in range(nu + npri, w0)]   # pad rows -> memset 0
    r = 0
    while r < ones_row:
        if srccol[r] is None:
            r += 1
            continue
        s = srccol[r]
        ln = 1
        while r + ln < ones_row and srccol[r + ln] == s + ln:
            ln += 1
        runs.append((r, s, ln))
        r += ln

    pri_rows = [(nu + i, frames[n]["src"], frames[n]["mult"], frames[n]["off"])
                for i, n in enumerate(pri)]
    consts = np.zeros((P, 4), np.float32)
    consts[:, 0] = mult_arr
    consts[:, 1] = off_arr
    consts[:, 2] = np.pi / 2.0
    return dict(K=K, nu=nu, npri=npri, w0=w0, ones_row=ones_row,
                runs=runs, pri_rows=pri_rows, zero_rows=zero_rows,
                consts=consts, table=table.astype(np.float16), tcols=tcols,
                kinds=[f["kind"] for f in frames])


# --------------------------------------------------------------------------
# Device program
# --------------------------------------------------------------------------

def _build_program(b_core, dof, nf, spec):
    assert b_core % P == 0
    q = b_core // P
    K = spec["K"]
    nu, w0, ones_row = spec["nu"], spec["w0"], spec["ones_row"]
    assert nf % CHUNK == 0
    reps = int(os.environ.get("FK_REPS", "1"))

    nc = bacc.Bacc("TRN2", target_bir_lowering=False, debug=False)

    theta_d = nc.dram_tensor("theta", [b_core, dof], F32,
                             kind="ExternalInput").ap()
    table_d = nc.dram_tensor("table", [P, nf * 12], F16,
                             kind="ExternalInput").ap()
    consts_d = nc.dram_tensor("consts", [P, 4], F32,
                              kind="ExternalInput").ap()
    # output layout [p, n, e, q] (q innermost): whole staging chunks DMA out
    # as one contiguous 16KB run per partition; host permutes to [b, n, e].
    out_d = nc.dram_tensor("out", [P, nf * 16 * q], F16,
                           kind="ExternalOutput").ap()

    theta_v = theta_d.rearrange("(p q) d -> p q d", p=P)

    from contextlib import ExitStack

    with tile.TileContext(nc) as tc, ExitStack() as ctx:
        pool = ctx.enter_context(tc.tile_pool(name="persist", bufs=1))
        tpp = ctx.enter_context(tc.tile_pool(name="tp_psum", bufs=2,
                                             space=bass.MemorySpace.PSUM))
        mmp = ctx.enter_context(tc.tile_pool(name="mm_psum", bufs=3,
                                             space=bass.MemorySpace.PSUM))
        mpool = ctx.enter_context(tc.tile_pool(name="mpool", bufs=4))

        QW = q // 4   # batch-column quarter for the pipelined head
        theta_t = pool.tile([P, q, dof], F32)
        for h in range(4):
            nc.sync.dma_start(theta_t[:, h * QW:(h + 1) * QW, :],
                              theta_v[:, h * QW:(h + 1) * QW, :])
        table_t = pool.tile([P, nf * 12], F16)
        nc.sync.dma_start(table_t[:], table_d)
        consts_t = pool.tile([P, 4], F32)
        nc.sync.dma_start(consts_t[:], consts_d)

        ident = pool.tile([P, P], F16)
        masks.make_identity(nc, ident[:])

        theta_dup = pool.tile([P, q, K], F16)
        coef_t = pool.tile([P, q, P], F16)     # [K rows, qq, 128]
        l_t = pool.tile([P, nf, 12, q], F16)

        stags = [pool.tile([P, CHUNK, 16, q], F16, tag=f"stag{i}",
                           name=f"stag{i}") for i in range(2)]
        for st in stags:
            nc.vector.memset(st[:, :, 12:15, :], 0.0)
            nc.vector.memset(st[:, :, 15, :], 1.0)

        # frame-groups for the L matmuls: small leading groups so the chain
        # starts early, 16-frame groups at steady state
        groups = []
        f0 = 0
        for sz in ([CHUNK, CHUNK] + [FG] * nf):
            if f0 >= nf:
                break
            sz = min(sz, nf - f0)
            groups.append((f0, sz))
            f0 += sz

        for _rep in range(reps):
            # ---- head, pipelined by batch-column quarters ----------------
            for h in range(4):
                qs_ = slice(h * QW, (h + 1) * QW)
                # theta_dup: batch-major x columns (dup for w rows); GpSimd
                for dst0, src0, ln in spec["runs"]:
                    nc.gpsimd.tensor_copy(theta_dup[:, qs_, dst0:dst0 + ln],
                                          theta_t[:, qs_, src0:src0 + ln])
                for r, s, m, o in spec["pri_rows"]:
                    nc.vector.tensor_scalar(theta_dup[:, qs_, r],
                                            theta_t[:, qs_, s],
                                            float(m), float(o),
                                            op0=OP.mult, op1=OP.add)
                for r0 in spec["zero_rows"]:
                    nc.vector.memset(theta_dup[:, qs_, r0], 0.0)
                nc.vector.memset(theta_dup[:, qs_, ones_row], 1.0)

                # transpose to s-major coef tile; 4 per PSUM tile, one DVE
                # evac per 4 columns (DVE is idle in the head)
                for qq0 in range(h * QW, (h + 1) * QW, 4):
                    ps = tpp.tile([K, 4, P], F16, tag="tp")
                    for j in range(4):
                        nc.tensor.transpose(ps[:, j, :],
                                            theta_dup[:, qq0 + j, :],
                                            ident[:])
                    nc.vector.tensor_copy(coef_t[:K, qq0:qq0 + 4, :], ps[:])

                # u rows: sin(mult*theta+off); w rows: sin(pi/2 - |m*x+o|)
                cv = coef_t[:, qs_, :].rearrange("p qq m -> p (qq m)")
                nc.scalar.activation(cv[0:nu], cv[0:nu], AF.Sin,
                                     bias=consts_t[0:nu, 1:2],
                                     scale=consts_t[0:nu, 0:1])
                nc.scalar.activation(cv[w0:w0 + nu], cv[w0:w0 + nu], AF.Abs,
                                     bias=consts_t[w0:w0 + nu, 1:2],
                                     scale=consts_t[w0:w0 + nu, 0:1])
                nc.scalar.activation(cv[w0:w0 + nu], cv[w0:w0 + nu], AF.Sin,
                                     bias=consts_t[w0:w0 + nu, 2:3],
                                     scale=-1.0)

            # ---- L via PE: [K,128] coef col x [K, 12*sz] table group -----
            for gi, (gf0, gsz) in enumerate(groups):
                gcol = slice(12 * gf0, 12 * (gf0 + gsz))
                for qq0 in range(0, q, QPAIR):
                    pmm = mmp.tile([P, QPAIR, 12 * FG], F32, tag="mm")
                    for j in range(QPAIR):
                        nc.tensor.matmul(pmm[:, j, :12 * gsz],
                                         coef_t[:K, qq0 + j, :],
                                         table_t[:K, gcol],
                                         start=True, stop=True)
                    # evac PSUM -> L fp16 [P, gsz, 12, QPAIR]; the first two
                    # groups alternate DVE/ScalarE (both idle pre-chain)
                    dst = l_t[:, gf0:gf0 + gsz, :, qq0:qq0 + QPAIR]
                    src = pmm[:, :, :12 * gsz] \
                        .rearrange("p j (f e) -> p j f e", e=12) \
                        .transpose([0, 2, 3, 1])
                    if gi == 0 and (qq0 // QPAIR) % 2 == 0:
                        nc.vector.tensor_copy(dst, src)
                    else:
                        nc.scalar.copy(dst, src)

            # ---- chain -------------------------------------------------
            def lrow(n, k, nj):
                # L_n row k broadcast over i: [P, 3, nj, q]
                return l_t[:, n, 4 * k:4 * k + nj, :].unsqueeze(1) \
                    .broadcast_to([P, 3, nj, q])

            def stag_view(ci, c):
                return stags[ci][:, c, :, :] \
                    .rearrange("p (i j) q -> p i j q", j=4)

            def tt(op, out, a, b):
                if QS >= q:
                    getattr(nc.vector, op)(out, a, b)
                else:
                    getattr(nc.vector, op)(out[..., :QS], a[..., :QS],
                                           b[..., :QS])
                    getattr(nc.gpsimd, op)(out[..., QS:], a[..., QS:],
                                           b[..., QS:])

            prev = None
            for n in range(nf):
                ci, c = (n // CHUNK) % 2, n % CHUNK
                out_f = stag_view(ci, c)     # [P, 4, 4, q]

                if prev is None:
                    nc.vector.tensor_copy(
                        out_f[:, :3, :, :],
                        l_t[:, n, :, :].rearrange("p (k j) q -> p k j q", j=4))
                    prev = (ci, c)
                    continue

                # uniform 6-op product: out[i,j] = sum_k W[i,k] L[k,j]
                # (j=3 column of L carries the frame's t-col), then
                # out[:, 3] += W t-col for the implicit bottom row of L.
                w_v = stag_view(*prev)

                def wcol4(k, w_v=w_v):
                    return w_v[:, :3, k, :].unsqueeze(2) \
                        .broadcast_to([P, 3, 4, q])

                p0 = mpool.tile([P, 3, 4, q], F16, tag="p0")
                p1 = mpool.tile([P, 3, 4, q], F16, tag="p1")
                tt("tensor_mul", p0[:], wcol4(0), lrow(n, 0, 4))
                tt("tensor_mul", p1[:], wcol4(1), lrow(n, 1, 4))
                tt("tensor_add", p0[:], p0[:], p1[:])
                tt("tensor_mul", p1[:], wcol4(2), lrow(n, 2, 4))
                tt("tensor_add", out_f[:, :3, :, :], p0[:], p1[:])
                tt("tensor_add", out_f[:, :3, 3, :],
                   out_f[:, :3, 3, :], w_v[:, :3, 3, :])

                prev = (ci, c)

                if c == CHUNK - 1:
                    g = n // CHUNK
                    blk = CHUNK * 16 * q
                    src = stags[ci][:].rearrange("p c e q -> p (c e q)")
                    dst = out_d[:, g * blk:(g + 1) * blk]
                    nc.sync.dma_start(dst, src)

    nc.compile()
    return nc


def _get_program(b_core, dof, nf, spec_key, spec):
    key = (b_core, dof, nf, os.environ.get("FK_REPS", "1"), QS, spec_key)
    prog = _program_cache.get(key)
    if prog is None:
        prog = _build_program(b_core, dof, nf, spec)
        _program_cache[key] = prog
    return prog


# --------------------------------------------------------------------------
# Entry point
# --------------------------------------------------------------------------

def kernel(joint_angles, all_axes, all_origins, mimic_multipliers,
           mimic_offsets, ctrlable_indices, mimic_dst_indices,
           mimic_src_indices, joint_types):
    global last_results, last_in_maps

    theta = np.ascontiguousarray(np.asarray(joint_angles, dtype=np.float32))
    batch, dof = theta.shape
    nf = np.asarray(all_axes).shape[0]

    frames = _frame_specs(
        all_axes, all_origins, mimic_multipliers, mimic_offsets,
        ctrlable_indices, mimic_dst_indices, mimic_src_indices, joint_types)
    spec = _host_spec(frames)
    spec_key = (spec["K"], spec["nu"], spec["npri"],
                tuple(spec["kinds"]), tuple(spec["runs"]),
                tuple(spec["pri_rows"]),
                spec["table"].tobytes(), spec["consts"].tobytes(),
                tuple(t if t is None else tuple(t) for t in spec["tcols"]))

    n_cores = N_CORES
    assert batch % n_cores == 0
    b_core = batch // n_cores

    nc = _get_program(b_core, dof, nf, spec_key, spec)

    in_maps = []
    for i in range(n_cores):
        in_maps.append({
            "theta": np.ascontiguousarray(theta[i * b_core:(i + 1) * b_core]),
            "table": np.ascontiguousarray(spec["table"]),
            "consts": np.ascontiguousarray(spec["consts"]),
        })
    last_in_maps = in_maps

    res = run_bass_kernel_spmd(nc, in_maps, core_ids=list(range(n_cores)))
    last_results = res

    q = b_core // P
    parts = []
    for i in range(n_cores):
        o = res.results[i]["out"].reshape(P, nf, 16, q)
        # [p, n, e, q] -> [p, q, n, e]; core-local batch b = p*q + qq
        parts.append(np.transpose(o, (0, 3, 1, 2)).reshape(b_core, nf, 16))
    out = np.concatenate(parts, axis=0)
    return out.astype(np.float32).reshape(batch, nf, 4, 4)
